# revision 1
# baseline (speedup 1.0000x reference)
"""FBP single-view backprojection kernel for Trainium2 (8 NeuronCores).

Strategy (V1):
  - View-sharded: core c handles views [c*64, (c+1)*64). No communication.
  - Ramp filter as a PE matmul: pf[(b,v), d_out] = sum_k projT[k,(b,v)] * FwR[k,d_out]
    where FwR[k,m] = filt[m-k+511]*w[k] is built on the host from the actual
    filt/w inputs (Toeplitz of the shipped filter times the pre-weight).
  - Backprojection gather: geometry is static, so per-pixel detector indices
    (int16) and interpolation weights W0/W1 = wgt*(1-frac), wgt*frac are
    precomputed on the host. Out-of-range taps are handled by a zero-padded
    detector table (no masks needed).
  - The gather itself runs on GPSIMD ap_gather over a pair-interleaved table
    T2[e] = [P[e], P[e+1]] (d=2), one gather per (pixel, batch); 8 views are
    processed per instruction (one per Q7 core group of 16 partitions).
  - Combine on DVE: out_b = W0*q0 + W1*q1, then strided DMA to the output.
"""
import sys
import numpy as np
from contextlib import ExitStack

sys.path.insert(0, "/opt/trn_rl_repo")

import concourse.bass as bass
import concourse.bacc as bacc
import concourse.mybir as mybir
from concourse.tile import TileContext

# ---------------- problem constants (hardcoded from the task spec) -----------
VIEWS = 512
DETS = 512
H_IMG = 256
W_IMG = 256
D_IMG = 0.006641
D_DET = 0.0072
ANG0 = 0.0
D_ANG = 2.0 * np.pi / VIEWS
S2R = 5.95
D2R = 4.906
VIRDET = D_DET * S2R / (S2R + D2R)

N_CORES = 8
VPC = VIEWS // N_CORES          # views per core = 64
NPIX = H_IMG * W_IMG            # 65536
NGROUPS = VPC // 8              # 8 gather rounds (8 views each)
CHUNK = 1024                    # pixels per ap_gather instruction
NCHUNK = NPIX // CHUNK          # 64
C2 = 16                         # chunks per staging tile (16*8 = 128 rows)
NST = NCHUNK // C2              # 4 stagings per group
NELEMS = 768                    # padded table entries (pairs)
OFF = 128                       # detector index offset into padded table

F32 = mybir.dt.float32
I16 = mybir.dt.int16


# ---------------- host-side static geometry ---------------------------------
def _geometry(v_lo, v_hi):
    """j0 (int16 padded index) and W0/W1 (f32) for views [v_lo, v_hi)."""
    betas = ANG0 + D_ANG * np.arange(VIEWS, dtype=np.float64)[v_lo:v_hi]
    cb = np.cos(betas)[:, None, None]
    sb = np.sin(betas)[:, None, None]
    xs = ((np.arange(W_IMG, dtype=np.float64) - (W_IMG - 1) / 2.0) * D_IMG)[None, None, :]
    ys = (((H_IMG - 1) / 2.0 - np.arange(H_IMG, dtype=np.float64)) * D_IMG)[None, :, None]
    d = S2R - (xs * cb + ys * sb)
    u = S2R * (ys * cb - xs * sb) / d
    wgt = (S2R / d) ** 2
    t = u / VIRDET + (DETS - 1) / 2.0
    i0 = np.floor(t)
    frac = t - i0
    j0 = (i0 + OFF).astype(np.int16)
    assert j0.min() >= 0 and j0.max() < NELEMS - 1, (j0.min(), j0.max())
    W0 = (wgt * (1.0 - frac)).astype(np.float32)
    W1 = (wgt * frac).astype(np.float32)
    nv = v_hi - v_lo
    return (j0.reshape(nv, NPIX), W0.reshape(nv, NPIX), W1.reshape(nv, NPIX))


def _host_static():
    """Per-core IDX and W tables in device layout."""
    IDX = np.empty((N_CORES, NGROUPS, 128, NCHUNK * (CHUNK // 16)), dtype=np.int16)
    W0d = np.empty((N_CORES, NGROUPS, NST, 128, CHUNK), dtype=np.float32)
    W1d = np.empty_like(W0d)
    for c in range(N_CORES):
        j0, W0, W1 = _geometry(c * VPC, (c + 1) * VPC)
        # IDX[g, 16k+p, chunk*(CHUNK//16) + s] = j0[g*8+k, chunk*CHUNK + s*16 + p]
        a = j0.reshape(NGROUPS, 8, NCHUNK, CHUNK // 16, 16)
        IDX[c] = a.transpose(0, 1, 4, 2, 3).reshape(NGROUPS, 128, NCHUNK * (CHUNK // 16))
        # W[g, st, c2*8+k, col] = W[g*8+k, (st*16+c2)*CHUNK + col]
        for W, Wd in ((W0, W0d), (W1, W1d)):
            b = W.reshape(NGROUPS, 8, NST, C2, CHUNK)
            Wd[c] = b.transpose(0, 2, 3, 1, 4).reshape(NGROUPS, NST, 128, CHUNK)
    return IDX, W0d, W1d


_STATIC_CACHE = {}


def host_static():
    if "s" not in _STATIC_CACHE:
        _STATIC_CACHE["s"] = _host_static()
    return _STATIC_CACHE["s"]


# ---------------- bass program ----------------------------------------------
def build_program(ngroups=NGROUPS, nst=NST, debug=False):
    nc = bacc.Bacc("TRN2", target_bir_lowering=False)
    projT = nc.dram_tensor("projT", [128, 4 * 128], F32, kind="ExternalInput")
    FwR = nc.dram_tensor("FwR", [128, 4 * DETS], F32, kind="ExternalInput")
    IDX = nc.dram_tensor("IDX", [NGROUPS, 128, NCHUNK * (CHUNK // 16)], I16, kind="ExternalInput")
    W0d = nc.dram_tensor("W0d", [NGROUPS, NST, 128, C2 * 8 * CHUNK // 128], F32, kind="ExternalInput")
    W1d = nc.dram_tensor("W1d", [NGROUPS, NST, 128, C2 * 8 * CHUNK // 128], F32, kind="ExternalInput")
    out = nc.dram_tensor("out", [2, VPC, NPIX], F32, kind="ExternalOutput")
    if debug:
        dbg_pf = nc.dram_tensor("dbg_pf", [128, DETS], F32, kind="ExternalOutput")
        dbg_t2 = nc.dram_tensor("dbg_t2", [128, 2 * NELEMS], F32, kind="ExternalOutput")
        dbg_g = nc.dram_tensor("dbg_g", [2, 128, 2 * CHUNK], F32, kind="ExternalOutput")
        dbg_s = nc.dram_tensor("dbg_s", [2, 128, 2 * CHUNK], F32, kind="ExternalOutput")

    ex = ExitStack()
    with TileContext(nc) as tc:
        with (
            tc.tile_pool(name="const", bufs=1) as cpool,
            tc.tile_pool(name="psum", bufs=1, space="PSUM") as ppool,
            tc.tile_pool(name="rep", bufs=2) as rpool,
            tc.tile_pool(name="gather", bufs=3) as gpool,
            tc.tile_pool(name="stage", bufs=2) as spool,
            tc.tile_pool(name="work", bufs=3) as wpool,
        ):
            # ---- filter: pf[(b,v), m] via PE ----
            projT_sb = cpool.tile([128, 4 * 128], F32)
            FwR_sb = cpool.tile([128, 4 * DETS], F32)
            nc.sync.dma_start(out=projT_sb[:], in_=projT[:])
            nc.sync.dma_start(out=FwR_sb[:], in_=FwR[:])
            pf_ps = ppool.tile([128, DETS], F32)
            for kc in range(4):
                nc.tensor.matmul(
                    pf_ps[:],
                    lhsT=projT_sb[:, kc * 128:(kc + 1) * 128],
                    rhs=FwR_sb[:, kc * DETS:(kc + 1) * DETS],
                    start=(kc == 0), stop=(kc == 3),
                )
            # ---- padded table + batch-blocked remap + DVE quad expand ----
            tpad = cpool.tile([128, NELEMS + 2], F32)
            nc.vector.memset(tpad[:], 0.0)
            nc.vector.tensor_copy(out=tpad[:, OFF:OFF + DETS], in_=pf_ps[:])
            t2f = cpool.tile([64, 2 * (NELEMS + 2)], F32)
            for b in range(2):
                nc.sync.dma_start(
                    out=t2f[:, b * (NELEMS + 2):(b + 1) * (NELEMS + 2)],
                    in_=tpad[b * 64:(b + 1) * 64, :],
                )
            # t4[v, 4e+2b+t] = t2f[v, b*(NELEMS+2) + e + t]
            t4 = cpool.tile([64, 4 * NELEMS], F32)
            from concourse.ap import AP as _AP
            t4_ap = t4[:]
            dst = _AP(t4_ap.tensor, t4_ap.offset,
                      [list(t4_ap.ap[0]), [4, NELEMS], [2, 2], [1, 2]])
            t2_ap = t2f[:]
            src = _AP(t2_ap.tensor, t2_ap.offset,
                      [list(t2_ap.ap[0]), [1, NELEMS], [NELEMS + 2, 2], [1, 2]])
            nc.vector.tensor_copy(out=dst, in_=src)
            if debug:
                nc.sync.dma_start(out=dbg_pf[:], in_=tpad[:, OFF:OFF + DETS])
                pass

            for g in range(ngroups):
                # replicate the 8 views' quad tables across their 16-partition
                # groups (one broadcast DMA: in AP [8 rows][x16 repeat][3072])
                rep = rpool.tile([128, 4 * NELEMS], F32, tag="rep", name="rep")
                src = t4[g * 8:(g + 1) * 8, :]
                src = src.unsqueeze(1).broadcast_to([8, 16, 4 * NELEMS])
                nc.sync.dma_start(out=rep[:], in_=src)
                for st in range(nst):
                    stage = spool.tile([128, 4 * CHUNK], F32, tag="s", name="stage")
                    # all 16 chunks' indices in one DMA
                    idx_t = gpool.tile([128, C2 * CHUNK // 16], I16, tag="idx")
                    w16 = C2 * (CHUNK // 16)
                    nc.sync.dma_start(out=idx_t[:], in_=IDX[g, :, st * w16:(st + 1) * w16])
                    for c2 in range(C2):
                        gt = gpool.tile([128, 4 * CHUNK], F32, tag="g", name="gt")
                        nc.gpsimd.ap_gather(
                            out_ap=gt[:], in_ap=rep[:],
                            idxs_ap=idx_t[:, c2 * (CHUNK // 16):(c2 + 1) * (CHUNK // 16)],
                            channels=128, num_elems=NELEMS, d=4, num_idxs=CHUNK,
                        )
                        # extract the 8 useful rows (one per Q7 group)
                        nc.scalar.dma_start(
                            out=stage[c2 * 8:(c2 + 1) * 8, :],
                            in_=gt[0:128:16, :],
                        )
                        if debug and g == 0 and st == 0 and c2 == 0:
                            nc.sync.dma_start(out=dbg_g[0], in_=gt[:, 0:2 * CHUNK])
                            nc.sync.dma_start(out=dbg_g[1], in_=gt[:, 2 * CHUNK:])
                    if debug and g == 0 and st == 0:
                        nc.sync.dma_start(out=dbg_s[0], in_=stage[:, 0:2 * CHUNK])
                        nc.sync.dma_start(out=dbg_s[1], in_=stage[:, 2 * CHUNK:])
                    w0t = wpool.tile([128, CHUNK], F32, tag="w0")
                    w1t = wpool.tile([128, CHUNK], F32, tag="w1")
                    nc.sync.dma_start(out=w0t[:], in_=W0d[g, st])
                    nc.sync.dma_start(out=w1t[:], in_=W1d[g, st])
                    for b in range(2):
                        t_a = wpool.tile([128, CHUNK], F32, tag="ta")
                        t_o = wpool.tile([128, CHUNK], F32, tag="to")
                        nc.vector.tensor_mul(out=t_a[:], in0=w0t[:], in1=stage[:, 2 * b + 0:4 * CHUNK:4])
                        nc.vector.tensor_mul(out=t_o[:], in0=w1t[:], in1=stage[:, 2 * b + 1:4 * CHUNK:4])
                        nc.vector.tensor_add(out=t_o[:], in0=t_a[:], in1=t_o[:])
                        # rows r = c2*8+k -> out[b, g*8+k, (st*16+c2)*CHUNK + col]
                        dst = out[b, g * 8:(g + 1) * 8, st * C2 * CHUNK:(st * C2 + C2) * CHUNK]
                        dst = dst.rearrange("k (c2 col) -> c2 k col", col=CHUNK)
                        nc.sync.dma_start(out=dst, in_=t_o[:])
    return nc


# ---------------- host runner ------------------------------------------------
def _host_inputs(projection, w, filt):
    """Build per-core input maps."""
    filt = np.asarray(filt, dtype=np.float32).reshape(-1)
    w = np.asarray(w, dtype=np.float32).reshape(-1)
    proj = np.asarray(projection, dtype=np.float32)

    # FwR[k, m] = filt[m - k + 511] * w[k], chunked [128, kc*512 + m]
    k_idx = np.arange(DETS)
    m_idx = np.arange(DETS)
    Fmat = filt[k_idx[:, None] - m_idx[None, :] + DETS - 1] * w[:, None]  # [k, m]
    FwR_dev = Fmat.reshape(4, 128, DETS).transpose(1, 0, 2).reshape(128, 4 * DETS)
    FwR_dev = np.ascontiguousarray(FwR_dev, dtype=np.float32)

    IDX, W0d, W1d = host_static()
    in_maps = []
    for c in range(N_CORES):
        # projT[k, kc*128 + n], n = b*64 + vl, for views c*64+vl
        pv = proj[:, 0, c * VPC:(c + 1) * VPC, :]          # [2, 64, 512]
        pT = pv.reshape(2 * VPC, DETS).T                   # [512(k), 128(n)]
        pT = pT.reshape(4, 128, 128).transpose(1, 0, 2).reshape(128, 4 * 128)
        in_maps.append({
            "projT": np.ascontiguousarray(pT, dtype=np.float32),
            "FwR": FwR_dev,
            "IDX": IDX[c],
            "W0d": W0d[c],
            "W1d": W1d[c],
        })
    return in_maps


_PROGRAM_CACHE = {}


def kernel(projection, w, filt):
    try:
        import profhook  # registers NTFF hook; harmless if absent
    except Exception:
        pass
    from concourse.bass_utils import run_bass_kernel_spmd

    if "nc" not in _PROGRAM_CACHE:
        nc = build_program()
        nc.finalize()
        _PROGRAM_CACHE["nc"] = nc
    nc = _PROGRAM_CACHE["nc"]
    in_maps = _host_inputs(projection, w, filt)
    res = run_bass_kernel_spmd(nc, in_maps, core_ids=list(range(N_CORES)))
    outs = [r["out"] for r in res.results]  # each [2, 64, 65536]
    full = np.concatenate(outs, axis=1)     # [2, 512, 65536]
    return full.reshape(2, VIEWS, H_IMG, W_IMG).astype(np.float32)



# revision 2
# speedup vs baseline: 1.0045x; 1.0045x over previous
"""FBP single-view backprojection for Trainium2 — V4: D4-symmetry packed gather
with 2-pixel-per-index packing (d=4 table windows).

V3 finding: ap_gather is RD_CMD-latency bound (~124 cyc per 4-index request,
~26 ns/idx) and everything else hides under it. V4 halves the index count:
adjacent pixels in the stream share one gather index e=min(i0_a,i0_b); the
d=4 window (P[e..e+3]) covers both pixels' tap pairs (|i0_a-i0_b|<=2, host-
asserted). Tap selection folds into 4 static weight streams per pixel class
(even/odd stream position): out = sum_tau W_tau * g[4i+tau], with exactly two
nonzero W_tau per pixel.

Weights and output are bf16 (error ~0.3% rms, gate is 2e-2); gather stays f32.
"""
import sys
import numpy as np
from contextlib import ExitStack

sys.path.insert(0, "/opt/trn_rl_repo")

from ml_dtypes import bfloat16

# ---------------- problem constants ------------------------------------------
VIEWS = 512
DETS = 512
H_IMG = 256
W_IMG = 256
D_IMG = 0.006641
D_DET = 0.0072
D_ANG = 2.0 * np.pi / VIEWS
S2R = 5.95
D2R = 4.906
VIRDET = D_DET * S2R / (S2R + D2R)

N_CORES = 8
NPIX = H_IMG * W_IMG
OFF = 128
NELEMS = 768
CHUNK = 2752                    # pixels per instruction (pairs: 1376 idx)
NPAIR = CHUNK // 2
NI_PHASE = 12
NI = 2 * NI_PHASE               # 24 instructions per unit stream
STREAM = NI * CHUNK             # 66048
SPHASE = NI_PHASE * CHUNK       # 33024


# ---------------- orbit / slot structure -------------------------------------
def elem_view(vb, j_elem):
    if j_elem < 4:
        return (vb + 128 * j_elem) % 512
    return (-vb - 128 * (j_elem - 4)) % 512


def elem_reversed(j_elem):
    return j_elem >= 4


def sigma_positions(j_elem, h, w):
    k = j_elem % 4
    a, b = h, w
    for _ in range(k):
        a, b = 255 - b, a
    if j_elem >= 4:
        a = 255 - a
    return a, b


def _pixel_stream(g):
    """per-phase (vb, h, w) padded to SPHASE, pairs never crossing rows."""
    hh, ww = np.meshgrid(np.arange(H_IMG), np.arange(W_IMG), indexing="ij")
    if g < 63:
        vb = g + 1
        h_, w_ = hh.ravel(), ww.ravel()
        hA, wA = h_[:SPHASE], w_[:SPHASE]          # 129 full rows
        hB, wB = h_[SPHASE:], w_[SPHASE:]          # 127 rows
        npad = SPHASE - hB.size                    # 512, pad with last pixels
        hB = np.concatenate([hB, hB[-npad:]])
        wB = np.concatenate([wB, wB[-npad:]])
        return [(vb, hA, wA), (vb, hB, wB)]
    # special unit: orbit(0) top half; orbit(64) anti-triangle h+w<=255
    hA, wA = hh[:128].ravel(), ww[:128].ravel()    # 32768
    npad = SPHASE - hA.size                        # 256
    hA = np.concatenate([hA, hA[-npad:]])
    wA = np.concatenate([wA, wA[-npad:]])
    hB, wB = [], []
    for h in range(H_IMG):
        wid = 256 - h
        ws = list(range(wid))
        if wid % 2:
            ws.append(wid - 1)                     # duplicate last -> even row
        hB.extend([h] * len(ws))
        wB.extend(ws)
    hB = np.asarray(hB); wB = np.asarray(wB)
    assert hB.size == SPHASE, hB.size              # 32896 + 128 dups = 33024
    return [(0, hA, wA), (64, hB, wB)]


def geometry(vb):
    beta = D_ANG * vb
    cb, sb = np.cos(beta), np.sin(beta)
    xs = ((np.arange(W_IMG) - (W_IMG - 1) / 2.0) * D_IMG)[None, :]
    ys = (((H_IMG - 1) / 2.0 - np.arange(H_IMG)) * D_IMG)[:, None]
    d = S2R - (xs * cb + ys * sb)
    u = S2R * (ys * cb - xs * sb) / d
    wgt = (S2R / d) ** 2
    t = u / VIRDET + (DETS - 1) / 2.0
    i0 = np.floor(t)
    frac = t - i0
    W0 = (wgt * (1.0 - frac)).astype(np.float32)
    W1 = (wgt * frac).astype(np.float32)
    j0 = (i0 + OFF).astype(np.int32)
    assert j0.min() >= 0 and j0.max() < NELEMS - 1, (j0.min(), j0.max())
    return j0, W0, W1


def _host_static():
    """IDX [cores,128,NI*NPAIR/16] wrapped pair indices; W4 [cores,NI,128,8*NPAIR]
    bf16 weight streams (cls-major: [cls*4+tau, pair]); streams for unshard."""
    IDX = np.zeros((N_CORES, 128, NI, NPAIR // 16), dtype=np.int16)
    W4 = np.zeros((N_CORES, NI, 128, 8 * NPAIR), dtype=bfloat16)
    streams = {}
    for g in range(64):
        c, k = divmod(g, 8)
        phases = _pixel_stream(g)
        pinfo = []
        for ph, (vb, h_, w_) in enumerate(phases):
            j0, W0, W1 = geometry(vb)
            js = j0[h_, w_]                     # [SPHASE]
            W0s = W0[h_, w_].astype(np.float64)
            W1s = W1[h_, w_].astype(np.float64)
            ja = js[0::2]; jb = js[1::2]
            e = np.minimum(ja, jb)              # pair base index
            span = np.maximum(ja, jb) - e
            assert span.max() <= 2, (g, ph, span.max())
            # weight streams: Wq[cls, tau, pair]
            Wq = np.zeros((2, 4, SPHASE // 2), dtype=np.float64)
            for cls, (jc, W0c, W1c) in enumerate(
                    [(ja, W0s[0::2], W1s[0::2]), (jb, W0s[1::2], W1s[1::2])]):
                dlt = jc - e                    # 0..2
                for tau in range(4):
                    Wq[cls, tau] = W0c * (dlt == tau) + W1c * (dlt + 1 == tau)
            for i in range(NI_PHASE):
                t = ph * NI_PHASE + i
                psl = slice(i * NPAIR, (i + 1) * NPAIR)
                IDX[c, 16 * k:16 * k + 16, t, :] = \
                    e[psl].reshape(NPAIR // 16, 16).T
                W4[c, t, 16 * k:16 * k + 16, :] = \
                    Wq[:, :, psl].reshape(8 * NPAIR).astype(bfloat16)[None, :]
            pinfo.append((vb, h_, w_))
        streams[(c, k)] = pinfo
    return IDX, W4, streams


_STATIC_CACHE = {}


def host_static():
    if "s" not in _STATIC_CACHE:
        _STATIC_CACHE["s"] = _host_static()
    return _STATIC_CACHE["s"]


# ---------------- per-call host inputs ---------------------------------------
def _host_inputs(projection, w, filt):
    filt = np.asarray(filt, dtype=np.float32).reshape(-1)
    w = np.asarray(w, dtype=np.float32).reshape(-1)
    proj = np.asarray(projection, dtype=np.float32)[:, 0]
    assert np.array_equal(filt, filt[::-1]), "filt must be symmetric"
    assert np.array_equal(w, w[::-1]), "w must be symmetric"

    k_idx = np.arange(DETS)
    m_idx = np.arange(DETS)
    Fmat = filt[k_idx[:, None] - m_idx[None, :] + DETS - 1] * w[:, None]
    FwR_dev = Fmat.reshape(4, 128, DETS).transpose(1, 0, 2).reshape(128, 4 * DETS)
    FwR_dev = np.ascontiguousarray(FwR_dev, dtype=np.float32)

    IDX, W4, streams = host_static()
    in_maps = []
    for c in range(N_CORES):
        cols = np.empty((2, 128, DETS), dtype=np.float32)
        for st in range(2):
            for k7 in range(8):
                g = c * 8 + k7
                vb = streams[(c, k7)][st][0]
                for j in range(16):
                    je, b = divmod(j, 2)
                    v = elem_view(vb, je)
                    row = proj[b, v]
                    if elem_reversed(je):
                        row = row[::-1]
                    cols[st, 16 * k7 + j] = row
        pT = cols.transpose(2, 0, 1).reshape(DETS, 2 * 128)
        pT = pT.reshape(4, 128, 2, 128).transpose(1, 2, 0, 3).reshape(128, 2 * 4 * 128)
        in_maps.append({
            "projT": np.ascontiguousarray(pT, dtype=np.float32),
            "FwR": FwR_dev,
            "IDX": np.ascontiguousarray(IDX[c].reshape(128, NI * (NPAIR // 16))),
            "W4d": W4[c],
        })
    return in_maps


# ---------------- host-side emulation of the device program ------------------
def emulate_device(in_maps):
    outs = []
    for c in range(N_CORES):
        m = in_maps[c]
        projT = m["projT"].reshape(128, 2, 4, 128)
        FwR = m["FwR"].reshape(128, 4, DETS)
        phat = np.zeros((2, 128, DETS), dtype=np.float32)
        for st in range(2):
            for kc in range(4):
                phat[st] += projT[:, st, kc, :].T @ FwR[:, kc, :]
        tpad = np.zeros((2, 128, NELEMS + 3), dtype=np.float32)
        tpad[:, :, OFF:OFF + DETS] = phat
        T = np.stack([tpad[:, :, tau:NELEMS + tau] for tau in range(4)], axis=-1)
        out = np.zeros((8, 16, STREAM), dtype=bfloat16)
        for t in range(NI):
            st = t // NI_PHASE
            idx = m["IDX"].reshape(128, NI, NPAIR // 16)[:, t]
            W4 = m["W4d"][t].astype(np.float32)     # [128, 8*NPAIR]
            for k7 in range(8):
                e = idx[16 * k7:16 * k7 + 16, :].T.reshape(NPAIR)
                for j in range(16):
                    p = 16 * k7 + j
                    Tj = T[st, p]                   # [NELEMS, 4]
                    G = Tj[e]                       # [NPAIR, 4]
                    Wq = W4[p].reshape(2, 4, NPAIR)
                    vals = np.empty(CHUNK, dtype=np.float32)
                    for cls in range(2):
                        acc = np.zeros(NPAIR, dtype=np.float32)
                        for tau in range(4):
                            acc += Wq[cls, tau] * G[:, tau]
                        vals[cls::2] = acc
                    out[k7, j, t * CHUNK:(t + 1) * CHUNK] = vals.astype(bfloat16)
        outs.append(out)
    return outs


# ---------------- unshard -----------------------------------------------------
def unshard(outs):
    _, _, streams = host_static()
    full = np.zeros((2, VIEWS, H_IMG, W_IMG), dtype=np.float32)
    for c in range(N_CORES):
        for k7 in range(8):
            pinfo = streams[(c, k7)]
            for ph, (vb, h_, w_) in enumerate(pinfo):
                sl = slice(ph * SPHASE, (ph + 1) * SPHASE)
                for j in range(16):
                    je, b = divmod(j, 2)
                    v = elem_view(vb, je)
                    vals = np.asarray(outs[c][k7, j, sl], dtype=np.float32)
                    ph_, pw_ = sigma_positions(je, h_, w_)
                    full[b, v, ph_, pw_] = vals
    return full


# ---------------- bass program ------------------------------------------------
def build_program():
    import concourse.bass as bass  # noqa
    import concourse.bacc as bacc
    import concourse.mybir as mybir
    from concourse.tile import TileContext
    from concourse.ap import AP as _AP

    F32 = mybir.dt.float32
    BF16 = mybir.dt.bfloat16
    I16 = mybir.dt.int16

    nc = bacc.Bacc("TRN2", target_bir_lowering=False)
    projT = nc.dram_tensor("projT", [128, 2 * 4 * 128], F32, kind="ExternalInput")
    FwR = nc.dram_tensor("FwR", [128, 4 * DETS], F32, kind="ExternalInput")
    IDX = nc.dram_tensor("IDX", [128, NI * (NPAIR // 16)], I16, kind="ExternalInput")
    W4d = nc.dram_tensor("W4d", [NI, 128, 8 * NPAIR], BF16, kind="ExternalInput")
    out = nc.dram_tensor("out", [8, 16, STREAM], BF16, kind="ExternalOutput")

    with TileContext(nc) as tc:
        with (
            tc.tile_pool(name="const", bufs=1) as cpool,
            tc.tile_pool(name="psum", bufs=1, space="PSUM") as ppool,
            tc.tile_pool(name="gather", bufs=2) as gpool,
            tc.tile_pool(name="wt", bufs=2) as wpool,
            tc.tile_pool(name="work", bufs=2) as opool,
        ):
            projT_sb = cpool.tile([128, 2 * 4 * 128], F32)
            FwR_sb = cpool.tile([128, 4 * DETS], F32)
            idx_all = cpool.tile([128, NI * (NPAIR // 16)], I16)
            nc.sync.dma_start(out=projT_sb[:], in_=projT[:])
            nc.sync.dma_start(out=FwR_sb[:], in_=FwR[:])
            nc.sync.dma_start(out=idx_all[:], in_=IDX[:])
            t4 = []
            for st in range(2):
                pf_ps = ppool.tile([128, DETS], F32, tag=f"pf{st}")
                for kc in range(4):
                    nc.tensor.matmul(
                        pf_ps[:],
                        lhsT=projT_sb[:, (st * 4 + kc) * 128:(st * 4 + kc + 1) * 128],
                        rhs=FwR_sb[:, kc * DETS:(kc + 1) * DETS],
                        start=(kc == 0), stop=(kc == 3),
                    )
                tpad = cpool.tile([128, NELEMS + 3], F32, tag=f"tpad{st}")
                nc.vector.memset(tpad[:], 0.0)
                nc.vector.tensor_copy(out=tpad[:, OFF:OFF + DETS], in_=pf_ps[:])
                # quad expand: t4[p, 4e+tau] = tpad[p, e+tau]
                t4t = cpool.tile([128, 4 * NELEMS], F32, tag=f"t4{st}")
                dst_ap = t4t[:]
                dst = _AP(dst_ap.tensor, dst_ap.offset,
                          [list(dst_ap.ap[0]), [4, NELEMS], [1, 4]])
                src_ap = tpad[:]
                src = _AP(src_ap.tensor, src_ap.offset,
                          [list(src_ap.ap[0]), [1, NELEMS], [1, 4]])
                nc.vector.tensor_copy(out=dst, in_=src)
                t4.append(t4t)

            for t in range(NI):
                st = t // NI_PHASE
                wv = wpool.tile([128, 8 * NPAIR], BF16, tag="wv")
                nc.sync.dma_start(out=wv[:], in_=W4d[t])
                gt = gpool.tile([128, 4 * NPAIR], F32, tag="gt")
                nc.gpsimd.ap_gather(
                    out_ap=gt[:], in_ap=t4[st][:],
                    idxs_ap=idx_all[:, t * (NPAIR // 16):(t + 1) * (NPAIR // 16)],
                    channels=128, num_elems=NELEMS, d=4, num_idxs=NPAIR,
                )
                t_o = opool.tile([128, CHUNK], BF16, tag="to")
                for cls in range(2):
                    acc = opool.tile([128, NPAIR], F32, tag=f"acc{cls}")
                    prd = opool.tile([128, NPAIR], F32, tag=f"prd{cls}")
                    nc.vector.tensor_mul(
                        out=acc[:], in0=wv[:, (cls * 4 + 0) * NPAIR:(cls * 4 + 1) * NPAIR],
                        in1=gt[:, 0:4 * NPAIR:4])
                    for tau in range(1, 4):
                        nc.vector.tensor_mul(
                            out=prd[:],
                            in0=wv[:, (cls * 4 + tau) * NPAIR:(cls * 4 + tau + 1) * NPAIR],
                            in1=gt[:, tau:4 * NPAIR:4])
                        if tau < 3:
                            nc.vector.tensor_add(out=acc[:], in0=acc[:], in1=prd[:])
                        else:
                            nc.vector.tensor_add(
                                out=t_o[:, cls:CHUNK:2], in0=acc[:], in1=prd[:])
                dst = out[:, :, t * CHUNK:(t + 1) * CHUNK]
                nc.scalar.dma_start(out=dst, in_=t_o[:])
    return nc


_PROGRAM_CACHE = {}


def kernel(projection, w, filt):
    try:
        import profhook  # noqa
    except Exception:
        pass
    from concourse.bass_utils import run_bass_kernel_spmd

    if "nc" not in _PROGRAM_CACHE:
        nc = build_program()
        nc.finalize()
        _PROGRAM_CACHE["nc"] = nc
    nc = _PROGRAM_CACHE["nc"]
    in_maps = _host_inputs(projection, w, filt)
    res = run_bass_kernel_spmd(nc, in_maps, core_ids=list(range(N_CORES)))
    outs = [np.asarray(r["out"]) for r in res.results]
    return unshard(outs)


if __name__ == "__main__":
    import np_reference
    rng = np.random.default_rng(0)
    projection = rng.standard_normal((2, 1, VIEWS, DETS)).astype(np.float32)
    import reference
    inputs = reference.setup_inputs()
    w = np.asarray(inputs["w"])
    filt = np.asarray(inputs["filt"])
    exp = np_reference.reference_np(projection, w, filt)
    in_maps = _host_inputs(projection, w, filt)
    outs = emulate_device(in_maps)
    act = unshard(outs)
    rel = np.linalg.norm(act - exp) / np.linalg.norm(exp)
    print("emulate_device vs np_reference rel:", rel)


# revision 3
# speedup vs baseline: 1.2002x; 1.1948x over previous
"""FBP single-view backprojection for Trainium2 — V4: D4-symmetry packed gather
with 2-pixel-per-index packing (d=4 table windows).

V3 finding: ap_gather is RD_CMD-latency bound (~124 cyc per 4-index request,
~26 ns/idx) and everything else hides under it. V4 halves the index count:
adjacent pixels in the stream share one gather index e=min(i0_a,i0_b); the
d=4 window (P[e..e+3]) covers both pixels' tap pairs (|i0_a-i0_b|<=2, host-
asserted). Tap selection folds into 4 static weight streams per pixel class
(even/odd stream position): out = sum_tau W_tau * g[4i+tau], with exactly two
nonzero W_tau per pixel.

Weights, output, AND the gather tables/output are bf16 (emulated error ~3e-3,
gate is 2e-2). bf16 tables halve the gather's SBUF pops (d_u32 4->2) and make
the DVE's stride-4 reads 8-byte-granule aligned.
"""
import sys
import numpy as np
from contextlib import ExitStack

sys.path.insert(0, "/opt/trn_rl_repo")

from ml_dtypes import bfloat16

# ---------------- problem constants ------------------------------------------
VIEWS = 512
DETS = 512
H_IMG = 256
W_IMG = 256
D_IMG = 0.006641
D_DET = 0.0072
D_ANG = 2.0 * np.pi / VIEWS
S2R = 5.95
D2R = 4.906
VIRDET = D_DET * S2R / (S2R + D2R)

N_CORES = 8
NPIX = H_IMG * W_IMG
OFF = 128
NELEMS = 768
CHUNK = 2752                    # pixels per instruction (pairs: 1376 idx)
NPAIR = CHUNK // 2
NI_PHASE = 12
NI = 2 * NI_PHASE               # 24 instructions per unit stream
STREAM = NI * CHUNK             # 66048
SPHASE = NI_PHASE * CHUNK       # 33024


# ---------------- orbit / slot structure -------------------------------------
def elem_view(vb, j_elem):
    if j_elem < 4:
        return (vb + 128 * j_elem) % 512
    return (-vb - 128 * (j_elem - 4)) % 512


def elem_reversed(j_elem):
    return j_elem >= 4


def sigma_positions(j_elem, h, w):
    k = j_elem % 4
    a, b = h, w
    for _ in range(k):
        a, b = 255 - b, a
    if j_elem >= 4:
        a = 255 - a
    return a, b


def _pixel_stream(g):
    """per-phase (vb, h, w) padded to SPHASE, pairs never crossing rows."""
    hh, ww = np.meshgrid(np.arange(H_IMG), np.arange(W_IMG), indexing="ij")
    if g < 63:
        vb = g + 1
        h_, w_ = hh.ravel(), ww.ravel()
        hA, wA = h_[:SPHASE], w_[:SPHASE]          # 129 full rows
        hB, wB = h_[SPHASE:], w_[SPHASE:]          # 127 rows
        npad = SPHASE - hB.size                    # 512, pad with last pixels
        hB = np.concatenate([hB, hB[-npad:]])
        wB = np.concatenate([wB, wB[-npad:]])
        return [(vb, hA, wA), (vb, hB, wB)]
    # special unit: orbit(0) top half; orbit(64) anti-triangle h+w<=255
    hA, wA = hh[:128].ravel(), ww[:128].ravel()    # 32768
    npad = SPHASE - hA.size                        # 256
    hA = np.concatenate([hA, hA[-npad:]])
    wA = np.concatenate([wA, wA[-npad:]])
    hB, wB = [], []
    for h in range(H_IMG):
        wid = 256 - h
        ws = list(range(wid))
        if wid % 2:
            ws.append(wid - 1)                     # duplicate last -> even row
        hB.extend([h] * len(ws))
        wB.extend(ws)
    hB = np.asarray(hB); wB = np.asarray(wB)
    assert hB.size == SPHASE, hB.size              # 32896 + 128 dups = 33024
    return [(0, hA, wA), (64, hB, wB)]


def geometry(vb):
    beta = D_ANG * vb
    cb, sb = np.cos(beta), np.sin(beta)
    xs = ((np.arange(W_IMG) - (W_IMG - 1) / 2.0) * D_IMG)[None, :]
    ys = (((H_IMG - 1) / 2.0 - np.arange(H_IMG)) * D_IMG)[:, None]
    d = S2R - (xs * cb + ys * sb)
    u = S2R * (ys * cb - xs * sb) / d
    wgt = (S2R / d) ** 2
    t = u / VIRDET + (DETS - 1) / 2.0
    i0 = np.floor(t)
    frac = t - i0
    W0 = (wgt * (1.0 - frac)).astype(np.float32)
    W1 = (wgt * frac).astype(np.float32)
    j0 = (i0 + OFF).astype(np.int32)
    assert j0.min() >= 0 and j0.max() < NELEMS - 1, (j0.min(), j0.max())
    return j0, W0, W1


def _host_static():
    """IDX [cores,128,NI*NPAIR/16] wrapped pair indices; W4 [cores,NI,128,8*NPAIR]
    bf16 weight streams (cls-major: [cls*4+tau, pair]); streams for unshard."""
    IDX = np.zeros((N_CORES, 128, NI, NPAIR // 16), dtype=np.int16)
    W4 = np.zeros((N_CORES, NI, 128, 8 * NPAIR), dtype=bfloat16)
    streams = {}
    for g in range(64):
        c, k = divmod(g, 8)
        phases = _pixel_stream(g)
        pinfo = []
        for ph, (vb, h_, w_) in enumerate(phases):
            j0, W0, W1 = geometry(vb)
            js = j0[h_, w_]                     # [SPHASE]
            W0s = W0[h_, w_].astype(np.float64)
            W1s = W1[h_, w_].astype(np.float64)
            ja = js[0::2]; jb = js[1::2]
            e = np.minimum(ja, jb)              # pair base index
            span = np.maximum(ja, jb) - e
            assert span.max() <= 2, (g, ph, span.max())
            # weight streams: Wq[cls, tau, pair]
            Wq = np.zeros((2, 4, SPHASE // 2), dtype=np.float64)
            for cls, (jc, W0c, W1c) in enumerate(
                    [(ja, W0s[0::2], W1s[0::2]), (jb, W0s[1::2], W1s[1::2])]):
                dlt = jc - e                    # 0..2
                for tau in range(4):
                    Wq[cls, tau] = W0c * (dlt == tau) + W1c * (dlt + 1 == tau)
            for i in range(NI_PHASE):
                t = ph * NI_PHASE + i
                psl = slice(i * NPAIR, (i + 1) * NPAIR)
                IDX[c, 16 * k:16 * k + 16, t, :] = \
                    e[psl].reshape(NPAIR // 16, 16).T
                W4[c, t, 16 * k:16 * k + 16, :] = \
                    Wq[:, :, psl].reshape(8 * NPAIR).astype(bfloat16)[None, :]
            pinfo.append((vb, h_, w_))
        streams[(c, k)] = pinfo
    return IDX, W4, streams


_STATIC_CACHE = {}


def host_static():
    if "s" not in _STATIC_CACHE:
        _STATIC_CACHE["s"] = _host_static()
    return _STATIC_CACHE["s"]


# ---------------- per-call host inputs ---------------------------------------
def _host_inputs(projection, w, filt):
    filt = np.asarray(filt, dtype=np.float32).reshape(-1)
    w = np.asarray(w, dtype=np.float32).reshape(-1)
    proj = np.asarray(projection, dtype=np.float32)[:, 0]
    assert np.array_equal(filt, filt[::-1]), "filt must be symmetric"
    assert np.array_equal(w, w[::-1]), "w must be symmetric"

    k_idx = np.arange(DETS)
    m_idx = np.arange(DETS)
    Fmat = filt[k_idx[:, None] - m_idx[None, :] + DETS - 1] * w[:, None]
    FwR_dev = Fmat.reshape(4, 128, DETS).transpose(1, 0, 2).reshape(128, 4 * DETS)
    FwR_dev = np.ascontiguousarray(FwR_dev, dtype=np.float32)

    IDX, W4, streams = host_static()
    in_maps = []
    for c in range(N_CORES):
        cols = np.empty((2, 128, DETS), dtype=np.float32)
        for st in range(2):
            for k7 in range(8):
                g = c * 8 + k7
                vb = streams[(c, k7)][st][0]
                for j in range(16):
                    je, b = divmod(j, 2)
                    v = elem_view(vb, je)
                    row = proj[b, v]
                    if elem_reversed(je):
                        row = row[::-1]
                    cols[st, 16 * k7 + j] = row
        pT = cols.transpose(2, 0, 1).reshape(DETS, 2 * 128)
        pT = pT.reshape(4, 128, 2, 128).transpose(1, 2, 0, 3).reshape(128, 2 * 4 * 128)
        in_maps.append({
            "projT": np.ascontiguousarray(pT, dtype=np.float32),
            "FwR": FwR_dev,
            "IDX": np.ascontiguousarray(IDX[c].reshape(128, NI * (NPAIR // 16))),
            "W4d": W4[c],
        })
    return in_maps


# ---------------- host-side emulation of the device program ------------------
def emulate_device(in_maps):
    outs = []
    for c in range(N_CORES):
        m = in_maps[c]
        projT = m["projT"].reshape(128, 2, 4, 128)
        FwR = m["FwR"].reshape(128, 4, DETS)
        phat = np.zeros((2, 128, DETS), dtype=np.float32)
        for st in range(2):
            for kc in range(4):
                phat[st] += projT[:, st, kc, :].T @ FwR[:, kc, :]
        tpad = np.zeros((2, 128, NELEMS + 3), dtype=np.float32)
        tpad[:, :, OFF:OFF + DETS] = phat.astype(bfloat16).astype(np.float32)
        T = np.stack([tpad[:, :, tau:NELEMS + tau] for tau in range(4)], axis=-1)
        out = np.zeros((8, 16, STREAM), dtype=bfloat16)
        for t in range(NI):
            st = t // NI_PHASE
            idx = m["IDX"].reshape(128, NI, NPAIR // 16)[:, t]
            W4 = m["W4d"][t].astype(np.float32)     # [128, 8*NPAIR]
            for k7 in range(8):
                e = idx[16 * k7:16 * k7 + 16, :].T.reshape(NPAIR)
                for j in range(16):
                    p = 16 * k7 + j
                    Tj = T[st, p]                   # [NELEMS, 4]
                    G = Tj[e]                       # [NPAIR, 4]
                    Wq = W4[p].reshape(2, 4, NPAIR)
                    vals = np.empty(CHUNK, dtype=np.float32)
                    for cls in range(2):
                        acc = np.zeros(NPAIR, dtype=np.float32)
                        for tau in range(4):
                            acc += Wq[cls, tau] * G[:, tau]
                        vals[cls::2] = acc
                    out[k7, j, t * CHUNK:(t + 1) * CHUNK] = vals.astype(bfloat16)
        outs.append(out)
    return outs


# ---------------- unshard -----------------------------------------------------
def unshard(outs):
    _, _, streams = host_static()
    full = np.zeros((2, VIEWS, H_IMG, W_IMG), dtype=np.float32)
    for c in range(N_CORES):
        for k7 in range(8):
            pinfo = streams[(c, k7)]
            for ph, (vb, h_, w_) in enumerate(pinfo):
                sl = slice(ph * SPHASE, (ph + 1) * SPHASE)
                for j in range(16):
                    je, b = divmod(j, 2)
                    v = elem_view(vb, je)
                    vals = np.asarray(outs[c][k7, j, sl], dtype=np.float32)
                    ph_, pw_ = sigma_positions(je, h_, w_)
                    full[b, v, ph_, pw_] = vals
    return full


# ---------------- bass program ------------------------------------------------
def build_program():
    import concourse.bass as bass  # noqa
    import concourse.bacc as bacc
    import concourse.mybir as mybir
    from concourse.tile import TileContext
    from concourse.ap import AP as _AP

    F32 = mybir.dt.float32
    BF16 = mybir.dt.bfloat16
    I16 = mybir.dt.int16

    nc = bacc.Bacc("TRN2", target_bir_lowering=False)
    projT = nc.dram_tensor("projT", [128, 2 * 4 * 128], F32, kind="ExternalInput")
    FwR = nc.dram_tensor("FwR", [128, 4 * DETS], F32, kind="ExternalInput")
    IDX = nc.dram_tensor("IDX", [128, NI * (NPAIR // 16)], I16, kind="ExternalInput")
    W4d = nc.dram_tensor("W4d", [NI, 128, 8 * NPAIR], BF16, kind="ExternalInput")
    out = nc.dram_tensor("out", [8, 16, STREAM], BF16, kind="ExternalOutput")

    with TileContext(nc) as tc:
        with (
            tc.tile_pool(name="const", bufs=1) as cpool,
            tc.tile_pool(name="psum", bufs=1, space="PSUM") as ppool,
            tc.tile_pool(name="gather", bufs=3) as gpool,
            tc.tile_pool(name="wt", bufs=2) as wpool,
            tc.tile_pool(name="work", bufs=2) as opool,
        ):
            projT_sb = cpool.tile([128, 2 * 4 * 128], F32)
            FwR_sb = cpool.tile([128, 4 * DETS], F32)
            idx_all = cpool.tile([128, NI * (NPAIR // 16)], I16)
            nc.sync.dma_start(out=projT_sb[:], in_=projT[:])
            nc.sync.dma_start(out=FwR_sb[:], in_=FwR[:])
            nc.sync.dma_start(out=idx_all[:], in_=IDX[:])
            t4 = []
            for st in range(2):
                pf_ps = ppool.tile([128, DETS], F32, tag=f"pf{st}")
                for kc in range(4):
                    nc.tensor.matmul(
                        pf_ps[:],
                        lhsT=projT_sb[:, (st * 4 + kc) * 128:(st * 4 + kc + 1) * 128],
                        rhs=FwR_sb[:, kc * DETS:(kc + 1) * DETS],
                        start=(kc == 0), stop=(kc == 3),
                    )
                tpad = cpool.tile([128, NELEMS + 3], F32, tag=f"tpad{st}")
                nc.vector.memset(tpad[:], 0.0)
                nc.vector.tensor_copy(out=tpad[:, OFF:OFF + DETS], in_=pf_ps[:])
                # quad expand: t4[p, 4e+tau] = tpad[p, e+tau]
                t4t = cpool.tile([128, 4 * NELEMS], BF16, tag=f"t4{st}")
                dst_ap = t4t[:]
                dst = _AP(dst_ap.tensor, dst_ap.offset,
                          [list(dst_ap.ap[0]), [4, NELEMS], [1, 4]])
                src_ap = tpad[:]
                src = _AP(src_ap.tensor, src_ap.offset,
                          [list(src_ap.ap[0]), [1, NELEMS], [1, 4]])
                nc.vector.tensor_copy(out=dst, in_=src)
                t4.append(t4t)

            for t in range(NI):
                st = t // NI_PHASE
                wv = wpool.tile([128, 8 * NPAIR], BF16, tag="wv")
                nc.sync.dma_start(out=wv[:], in_=W4d[t])
                gt = gpool.tile([128, 4 * NPAIR], BF16, tag="gt")
                nc.gpsimd.ap_gather(
                    out_ap=gt[:], in_ap=t4[st][:],
                    idxs_ap=idx_all[:, t * (NPAIR // 16):(t + 1) * (NPAIR // 16)],
                    channels=128, num_elems=NELEMS, d=4, num_idxs=NPAIR,
                )
                t_o = opool.tile([128, CHUNK], BF16, tag="to")
                for cls in range(2):
                    acc = opool.tile([128, NPAIR], F32, tag=f"acc{cls}")
                    prd = opool.tile([128, NPAIR], F32, tag=f"prd{cls}")
                    nc.vector.tensor_mul(
                        out=acc[:], in0=wv[:, (cls * 4 + 0) * NPAIR:(cls * 4 + 1) * NPAIR],
                        in1=gt[:, 0:4 * NPAIR:4])
                    for tau in range(1, 4):
                        nc.vector.tensor_mul(
                            out=prd[:],
                            in0=wv[:, (cls * 4 + tau) * NPAIR:(cls * 4 + tau + 1) * NPAIR],
                            in1=gt[:, tau:4 * NPAIR:4])
                        if tau < 3:
                            nc.vector.tensor_add(out=acc[:], in0=acc[:], in1=prd[:])
                        else:
                            nc.vector.tensor_add(
                                out=t_o[:, cls:CHUNK:2], in0=acc[:], in1=prd[:])
                dst = out[:, :, t * CHUNK:(t + 1) * CHUNK]
                nc.scalar.dma_start(out=dst, in_=t_o[:])
    return nc


_PROGRAM_CACHE = {}


def kernel(projection, w, filt):
    try:
        import profhook  # noqa
    except Exception:
        pass
    from concourse.bass_utils import run_bass_kernel_spmd

    if "nc" not in _PROGRAM_CACHE:
        nc = build_program()
        nc.finalize()
        _PROGRAM_CACHE["nc"] = nc
    nc = _PROGRAM_CACHE["nc"]
    in_maps = _host_inputs(projection, w, filt)
    res = run_bass_kernel_spmd(nc, in_maps, core_ids=list(range(N_CORES)))
    outs = [np.asarray(r["out"]) for r in res.results]
    return unshard(outs)


if __name__ == "__main__":
    import np_reference
    rng = np.random.default_rng(0)
    projection = rng.standard_normal((2, 1, VIEWS, DETS)).astype(np.float32)
    import reference
    inputs = reference.setup_inputs()
    w = np.asarray(inputs["w"])
    filt = np.asarray(inputs["filt"])
    exp = np_reference.reference_np(projection, w, filt)
    in_maps = _host_inputs(projection, w, filt)
    outs = emulate_device(in_maps)
    act = unshard(outs)
    rel = np.linalg.norm(act - exp) / np.linalg.norm(exp)
    print("emulate_device vs np_reference rel:", rel)
